# revision 6
# baseline (speedup 1.0000x reference)
"""Two-layer residual GCN (PyG GCNConv-style) on 8 Trainium2 NeuronCores, v2.

Same dst-sharded skeleton as v1, tuned around the real bottleneck (GpSimd
descriptor generation for dma_gather, ~8 ns per gathered row, serial):

  - Self-loops are not materialized as edges: the epilogue adds the own-window
    table row (g_raw[d]*dinv[d]) before the final dinv[d] scale, giving
    dinv[d]^2*g_raw[d] exactly. Saves ~3% of gather descriptors.
  - Gather slots are sized per (chunk, window) cell to the max edge count over
    the 8 cores (SPMD program is shared), not rounded up to 128 per cell:
    saves ~5% descriptors. Blocks that straddle a window boundary get one
    masked one-hot pass per extra window.
  - dma_gather instructions rotate across 4 SWDGE queues: desc-gen for the
    next gather is not blocked behind the previous gather's ring drain.
  - Gathers are issued chunk-major so no gather waits on a later chunk's
    AllGather; per-window sums accumulate in an SBUF fp32 tile.
  - Tables, gathered messages, one-hots, and matmuls are bf16 (halves gather
    DMA bytes and AllGather wire, 2x LDWEIGHTS); the residual path (x, h1,
    accumulators, output) stays fp32.
  - dinv = 1/sqrt(deg+1) comes from the host.
"""

import os
import sys

import numpy as np

for _p in ("/opt/trn_rl_repo",):
    if _p not in sys.path and os.path.isdir(_p):
        sys.path.insert(0, _p)

from concourse import bacc, bass, mybir
from concourse.tile import TileContext

F32 = mybir.dt.float32
BF16 = mybir.dt.bfloat16
I16 = mybir.dt.int16

N_CORES = 8
N_CHUNKS = 4
GROUP_W = 4  # windows per gather unit
D = 128
QROT = int(os.environ.get("GCN_QROT", "4"))
SCRATCH = int(os.environ.get("GCN_SCRATCH", "49152"))


# --------------------------------------------------------------------------
# Planning (host): common SPMD structure + per-core data
# --------------------------------------------------------------------------
class Plan:
    def __init__(self, n_nodes, n_edges):
        self.N = n_nodes
        self.E = n_edges
        self.shard = -(-n_nodes // N_CORES)
        self.qrows = -(-(-(-self.shard // N_CHUNKS)) // 128) * 128
        self.SH = N_CHUNKS * self.qrows
        self.W = self.SH // 128
        self.WQ = self.qrows // 128
        self.TROWS = N_CORES * self.qrows
        assert self.TROWS <= 32767
        self.real_w = -(-self.shard // 128)


def _host_prepare(x, edge_index, W1, b1, W2, b2):
    import ml_dtypes

    N, d = x.shape
    assert d == D
    E = edge_index.shape[1]
    p = Plan(N, E)

    src = np.ascontiguousarray(edge_index[0]).astype(np.int64)
    dst = np.ascontiguousarray(edge_index[1]).astype(np.int64)

    core = dst // p.shard
    l_dst = dst - core * p.shard
    win = l_dst // 128
    dr = (l_dst % 128).astype(np.int64)
    r_src = src // p.shard
    l_src = src - r_src * p.shard
    q_src = l_src // p.qrows
    t_row = r_src * p.qrows + (l_src - q_src * p.qrows)

    # cell = (chunk, window); common size = max over cores
    ncell = N_CHUNKS * p.W
    cellid = q_src * p.W + win
    counts = np.zeros((N_CORES, ncell), dtype=np.int64)
    for c in range(N_CORES):
        counts[c] = np.bincount(cellid[core == c], minlength=ncell)
    L = counts.max(axis=0).reshape(N_CHUNKS, p.W)  # [chunk, window]

    # common slot layout: units = (chunk q, group g of GROUP_W windows)
    n_groups = -(-p.W // GROUP_W)
    units = []  # (q, icol0, n_slot, nblk, passes, windows)
    # passes: list of (blk, w, pcol, start, stop)
    wv_all = {}  # unit -> per-slot window id (-1 pad)
    cell_off = np.zeros((N_CHUNKS, p.W), dtype=np.int64)  # slot offset of cell
    icol = 0
    pcol = 0
    for q in range(N_CHUNKS):
        for g in range(n_groups):
            ws = list(range(g * GROUP_W, min((g + 1) * GROUP_W, p.W)))
            lens = [int(L[q, w]) for w in ws]
            tot = int(sum(lens))
            if tot == 0:
                units.append(None)
                continue
            n_slot = -(-tot // 128) * 128
            wv = np.full(n_slot, -1, dtype=np.int64)
            off = 0
            for w, ln in zip(ws, lens):
                cell_off[q, w] = icol * 16 + off
                wv[off : off + ln] = w
                off += ln
            nblk = -(-n_slot // 128)
            tmp = []
            for b in range(nblk):
                bw = wv[b * 128 : (b + 1) * 128]
                for w in np.unique(bw[bw >= 0]):
                    tmp.append((b, int(w), pcol))
                    pcol += 1
            first, last = {}, {}
            for b, w, pc in tmp:
                if w not in first:
                    first[w] = pc
                last[w] = pc
            passes = [
                (b, w, pc, pc == first[w], pc == last[w]) for b, w, pc in tmp
            ]
            units.append((q, icol, n_slot, nblk, passes, ws))
            wv_all[(q, g)] = wv
            icol += n_slot // 16
    p.units = units
    p.n_groups = n_groups
    p.TOTC = max(icol, 1)
    p.PASST = max(pcol, 1)
    p.MAXBLK = max((u[3] for u in units if u), default=1)
    p.MAXPASS = max((len(u[4]) for u in units if u), default=1)

    # degree incl self-loop -> dinv
    deg = np.bincount(dst, minlength=N).astype(np.float64) + 1.0
    dinv_full = (1.0 / np.sqrt(deg)).astype(np.float32)

    iota = np.tile(np.arange(128, dtype=np.float32), (128, 1)).astype(
        ml_dtypes.bfloat16
    )
    ident_bf = np.eye(128, dtype=np.float32).astype(ml_dtypes.bfloat16)
    b1t = np.tile(b1.astype(np.float32), (128, 1))
    b2t = np.tile(b2.astype(np.float32), (128, 1))

    in_maps = []
    for c in range(N_CORES):
        m = core == c
        ec, ew, et, edr = cellid[m], win[m], t_row[m], dr[m]
        order = np.argsort(ec, kind="stable")
        ec, ew, et, edr = ec[order], ew[order], et[order], edr[order]
        # slot position: cell offset + rank within cell
        cstart = np.zeros(ncell + 1, dtype=np.int64)
        np.cumsum(np.bincount(ec, minlength=ncell), out=cstart[1:])
        rank = np.arange(len(ec)) - cstart[ec]
        qq = ec // p.W
        wwin = ec % p.W
        slot = cell_off[qq, wwin] + rank

        idx_flat = np.zeros(p.TOTC * 16, dtype=np.int16)
        drel_slot = np.full(p.TOTC * 16, -1.0, dtype=np.float32)
        idx_flat[slot] = et.astype(np.int16)
        drel_slot[slot] = edr.astype(np.float32)

        # idx wrapped [16, TOTC] then replicated x8
        idx16 = idx_flat.reshape(p.TOTC, 16).T
        idx128 = np.tile(idx16, (8, 1))

        # per-pass drel columns [128, PASST]: block slots masked to the pass's
        # window via the common window layout
        drel = np.full((128, p.PASST), -1.0, dtype=np.float32)
        for u in units:
            if u is None:
                continue
            q, ic0, n_slot, nblk, passes, ws = u
            g = (ws[0]) // GROUP_W
            wv = wv_all[(q, g)]
            base = ic0 * 16
            for b, w, pc, st, sp in passes:
                s0 = b * 128
                s1 = min((b + 1) * 128, n_slot)
                seg_w = wv[s0:s1]
                seg_d = drel_slot[base + s0 : base + s1]
                col = np.where(seg_w == w, seg_d, -1.0).astype(np.float32)
                full = np.full(128, -1.0, dtype=np.float32)
                full[: len(col)] = col
                drel[:, pc] = full

        n0 = c * p.shard
        nreal = max(0, min(N - n0, p.shard))
        x_pad = np.zeros((p.SH, D), dtype=np.float32)
        x_pad[:nreal] = x[n0 : n0 + nreal]
        x_tiled = np.ascontiguousarray(x_pad.reshape(p.W, 128, D))
        xT = np.ascontiguousarray(x_pad.T).astype(ml_dtypes.bfloat16)

        dinv_pad = np.ones(p.SH, dtype=np.float32)
        dinv_pad[:nreal] = dinv_full[n0 : n0 + nreal]
        dinv_t = np.ascontiguousarray(dinv_pad.reshape(p.W, 128).T)

        in_maps.append(
            {
                "x_tiled": x_tiled,
                "xT": xT,
                "W1": W1.astype(ml_dtypes.bfloat16),
                "W2": W2.astype(ml_dtypes.bfloat16),
                "b1t": b1t,
                "b2t": b2t,
                "iota": iota,
                "ident_bf": ident_bf,
                "dinv": dinv_t,
                "idx16": idx128,
                "dstrel": drel.astype(ml_dtypes.bfloat16),
            }
        )
    return p, in_maps


# --------------------------------------------------------------------------
# Device program
# --------------------------------------------------------------------------
def _build_program(p: Plan):
    from contextlib import ExitStack

    nc = bacc.Bacc(
        "TRN2",
        target_bir_lowering=False,
        debug=False,
        num_devices=N_CORES,
        num_swdge_queues=max(QROT, 1),
        dynamic_dma_scratch_size=SCRATCH,
    )
    RG = [list(range(N_CORES))]

    x_tiled = nc.dram_tensor("x_tiled", [p.W, 128, D], F32, kind="ExternalInput")
    xT_d = nc.dram_tensor("xT", [D, p.SH], BF16, kind="ExternalInput")
    W1_d = nc.dram_tensor("W1", [D, D], BF16, kind="ExternalInput")
    W2_d = nc.dram_tensor("W2", [D, D], BF16, kind="ExternalInput")
    b1_d = nc.dram_tensor("b1t", [128, D], F32, kind="ExternalInput")
    b2_d = nc.dram_tensor("b2t", [128, D], F32, kind="ExternalInput")
    iota_d = nc.dram_tensor("iota", [128, 128], BF16, kind="ExternalInput")
    identb_d = nc.dram_tensor("ident_bf", [128, 128], BF16, kind="ExternalInput")
    dinv_d = nc.dram_tensor("dinv", [128, p.W], F32, kind="ExternalInput")
    idx_d = nc.dram_tensor("idx16", [128, p.TOTC], I16, kind="ExternalInput")
    drel_d = nc.dram_tensor("dstrel", [128, p.PASST], BF16, kind="ExternalInput")

    out_d = nc.dram_tensor("out", [p.W, 128, D], F32, kind="ExternalOutput")

    gq = [
        [nc.dram_tensor(f"g{l}q{q}", [p.qrows, D], BF16) for q in range(N_CHUNKS)]
        for l in (1, 2)
    ]
    tables = [
        [
            nc.dram_tensor(f"t{l}q{q}", [p.TROWS, D], BF16, addr_space="Shared")
            for q in range(N_CHUNKS)
        ]
        for l in (1, 2)
    ]
    h1_d = nc.dram_tensor("h1", [p.W, 128, D], F32)

    gq_i = 0  # rotating swdge queue counter

    with TileContext(nc) as tc:
        ctx = ExitStack()
        cst = ctx.enter_context(tc.tile_pool(name="cst", bufs=1))
        w1_sb = cst.tile([D, D], BF16, tag="w1")
        w2_sb = cst.tile([D, D], BF16, tag="w2")
        b1_sb = cst.tile([128, D], F32, tag="b1")
        b2_sb = cst.tile([128, D], F32, tag="b2")
        iota_sb = cst.tile([128, 128], BF16, tag="iota")
        identb_sb = cst.tile([128, 128], BF16, tag="identb")
        dinv_sb = cst.tile([128, p.W], F32, tag="dinv")
        for t, dr_ in (
            (w1_sb, W1_d),
            (w2_sb, W2_d),
            (b1_sb, b1_d),
            (b2_sb, b2_d),
            (iota_sb, iota_d),
            (identb_sb, identb_d),
            (dinv_sb, dinv_d),
        ):
            nc.sync.dma_start(out=t[:, :], in_=dr_[:, :])

        accum_pool = ctx.enter_context(tc.tile_pool(name="acc", bufs=1))
        accum = accum_pool.tile([128, p.W * 128], F32, tag="accum")

        def warmup():
            with tc.tile_pool(name="wu", bufs=1) as wp:
                widx = wp.tile([128, 1], I16, tag="widx")
                nc.vector.memset(widx[:, :], 0.0)
                wout = wp.tile([128, 1, 128], F32, tag="wout")
                nc.gpsimd.dma_gather(
                    out_ap=wout[:, :, :],
                    in_ap=x_tiled[0, :, :],
                    idxs_ap=widx[:, :],
                    num_idxs=16,
                    num_idxs_reg=16,
                    elem_size=D,
                    single_packet=False,
                    queue_num=0,
                )

        def g1_phase():
            with tc.tile_pool(name="xT", bufs=1) as xp, \
                 tc.tile_pool(name="g1o", bufs=4) as go, \
                 tc.tile_pool(name="g1p", bufs=2, space="PSUM") as gp:
                xT_sb = xp.tile([D, p.SH], BF16, tag="xT")
                nc.sync.dma_start(out=xT_sb[:, :], in_=xT_d[:, :])
                for q in range(N_CHUNKS):
                    for wq in range(p.WQ):
                        w = q * p.WQ + wq
                        ps = gp.tile([128, D], F32, tag="ps")
                        nc.tensor.matmul(
                            ps[:, :],
                            xT_sb[:, w * 128 : (w + 1) * 128],
                            w1_sb[:, :],
                            start=True,
                            stop=True,
                        )
                        gt = go.tile([128, D], BF16, tag="gt")
                        nc.scalar.activation(
                            gt[:, :], ps[:, :],
                            mybir.ActivationFunctionType.Copy,
                            scale=dinv_sb[:, w : w + 1],
                        )
                        nc.sync.dma_start(
                            out=gq[0][q][wq * 128 : (wq + 1) * 128, :],
                            in_=gt[:, :],
                        )
                    nc.gpsimd.collective_compute(
                        "AllGather",
                        mybir.AluOpType.bypass,
                        replica_groups=RG,
                        ins=[gq[0][q][:, :]],
                        outs=[tables[0][q][:, :]],
                    )

        epi1_ag_queue = []

        def agg_phase(layer, epi_cb=None):
            nonlocal gq_i
            table = tables[layer - 1]
            nc.vector.memset(accum[:, :], 0.0)
            pending_ags = epi1_ag_queue  # epi_cb appends here
            ag_ready = []
            with (
                tc.tile_pool(name=f"mt{layer}", bufs=4) as mp,
                tc.tile_pool(name=f"at{layer}", bufs=3) as ap_,
                tc.tile_pool(name=f"ix{layer}", bufs=3) as ip,
                tc.tile_pool(name=f"dl{layer}", bufs=3) as dp,
                tc.tile_pool(name=f"ps{layer}", bufs=4, space="PSUM") as pp,
            ):
                emitted = set()
                for u in p.units:
                    if u is None:
                        continue
                    # fire AGs queued >=2 units ago: their input DMAs have
                    # drained, so the trigger won't stall the gather stream
                    for ag in ag_ready:
                        ag()
                    del ag_ready[:]
                    ag_ready.extend(pending_ags)
                    del pending_ags[:]
                    q, ic0, n_slot, nblk, passes, ws = u
                    it = ip.tile([128, n_slot // 16], I16, tag="it")
                    nc.sync.dma_start(
                        out=it[:, :], in_=idx_d[:, ic0 : ic0 + n_slot // 16]
                    )
                    npass = len(passes)
                    dt_ = dp.tile([128, npass], BF16, tag="dl")
                    p0 = passes[0][2]
                    nc.sync.dma_start(
                        out=dt_[:, :], in_=drel_d[:, p0 : p0 + npass]
                    )
                    mt = mp.tile([128, p.MAXBLK, 128], BF16, tag="mt")
                    nc.gpsimd.dma_gather(
                        out_ap=mt[:, :nblk, :],
                        in_ap=table[q][:, :],
                        idxs_ap=it[:, :],
                        num_idxs=n_slot,
                        num_idxs_reg=n_slot,
                        elem_size=D,
                        single_packet=False,
                        queue_num=gq_i % max(QROT, 1),
                    )
                    gq_i += 1
                    at = ap_.tile([128, npass, 128], BF16, tag="at")
                    nc.vector.tensor_tensor(
                        at[:, :, :],
                        iota_sb.unsqueeze(1).broadcast_to([128, npass, 128]),
                        dt_.unsqueeze(2).broadcast_to([128, npass, 128]),
                        mybir.AluOpType.is_equal,
                    )
                    psums = {}
                    for b, w, pc, st, sp in passes:
                        if w not in psums:
                            psums[w] = pp.tile(
                                [128, D], F32, tag="ps", name=f"ps{layer}_{q}_{w}"
                            )
                        nc.tensor.matmul(
                            psums[w][:, :],
                            at[:, pc - p0, :],
                            mt[:, b, :],
                            start=st,
                            stop=sp,
                        )
                    for w in sorted(psums):
                        nc.vector.tensor_add(
                            accum[:, w * 128 : (w + 1) * 128],
                            accum[:, w * 128 : (w + 1) * 128],
                            psums[w][:, :],
                        )
                    # once the last chunk's partial sums for this window group
                    # are in, its epilogue can run under the remaining gathers
                    if epi_cb is not None and q == N_CHUNKS - 1:
                        for w in ws:
                            if w < p.real_w and w not in emitted:
                                epi_cb(w)
                                emitted.add(w)
                if epi_cb is not None:
                    for w in range(p.real_w):
                        if w not in emitted:
                            epi_cb(w)
                for ag in ag_ready + pending_ags:
                    ag()
                del ag_ready[:]
                del pending_ags[:]

        def emit_epi1(w, ep, p2):
            q, wq = w // p.WQ, w % p.WQ
            own = ep.tile([128, D], BF16, tag="own")
            nc.sync.dma_start(
                out=own[:, :], in_=gq[0][q][wq * 128 : (wq + 1) * 128, :]
            )
            xw = ep.tile([128, D], F32, tag="xw")
            nc.sync.dma_start(out=xw[:, :], in_=x_tiled[w, :, :])
            t1 = ep.tile([128, D], F32, tag="t1")
            nc.vector.tensor_add(
                t1[:, :], accum[:, w * 128 : (w + 1) * 128], own[:, :]
            )
            nc.scalar.activation(
                t1[:, :], t1[:, :], mybir.ActivationFunctionType.Copy,
                scale=dinv_sb[:, w : w + 1],
            )
            nc.vector.tensor_add(t1[:, :], t1[:, :], xw[:, :])
            nc.vector.tensor_add(t1[:, :], t1[:, :], b1_sb[:, :])
            h1w = ep.tile([128, D], F32, tag="h1w")
            nc.scalar.activation(
                h1w[:, :], t1[:, :], mybir.ActivationFunctionType.Relu
            )
            nc.sync.dma_start(out=h1_d[w, :, :], in_=h1w[:, :])
            h1b = ep.tile([128, D], BF16, tag="h1b")
            nc.vector.tensor_copy(h1b[:, :], h1w[:, :])
            pt = p2.tile([128, D], BF16, tag="pt")
            nc.tensor.transpose(pt[:, :], h1b[:, :], identb_sb[:, :])
            h1T = ep.tile([128, D], BF16, tag="h1T")
            nc.vector.tensor_copy(h1T[:, :], pt[:, :])
            pg = p2.tile([128, D], F32, tag="pg")
            nc.tensor.matmul(
                pg[:, :], h1T[:, :], w2_sb[:, :], start=True, stop=True
            )
            g2t = ep.tile([128, D], BF16, tag="g2t")
            nc.scalar.activation(
                g2t[:, :], pg[:, :], mybir.ActivationFunctionType.Copy,
                scale=dinv_sb[:, w : w + 1],
            )
            nc.sync.dma_start(
                out=gq[1][q][wq * 128 : (wq + 1) * 128, :], in_=g2t[:, :]
            )
            if wq == p.WQ - 1 or w == p.real_w - 1:
                if w == p.real_w - 1 and wq != p.WQ - 1:
                    z = ep.tile([128, D], BF16, tag="z")
                    nc.vector.memset(z[:, :], 0.0)
                    for wq2 in range(wq + 1, p.WQ):
                        nc.sync.dma_start(
                            out=gq[1][q][wq2 * 128 : (wq2 + 1) * 128, :],
                            in_=z[:, :],
                        )
                qq = q
                epi1_ag_queue.append(lambda: nc.gpsimd.collective_compute(
                    "AllGather",
                    mybir.AluOpType.bypass,
                    replica_groups=RG,
                    ins=[gq[1][qq][:, :]],
                    outs=[tables[1][qq][:, :]],
                ))

        def emit_epi2(w, ep):
            q, wq = w // p.WQ, w % p.WQ
            own = ep.tile([128, D], BF16, tag="own")
            nc.sync.dma_start(
                out=own[:, :], in_=gq[1][q][wq * 128 : (wq + 1) * 128, :]
            )
            h1w = ep.tile([128, D], F32, tag="h1w")
            nc.sync.dma_start(out=h1w[:, :], in_=h1_d[w, :, :])
            t1 = ep.tile([128, D], F32, tag="t1")
            nc.vector.tensor_add(
                t1[:, :], accum[:, w * 128 : (w + 1) * 128], own[:, :]
            )
            nc.scalar.activation(
                t1[:, :], t1[:, :], mybir.ActivationFunctionType.Copy,
                scale=dinv_sb[:, w : w + 1],
            )
            nc.vector.tensor_add(t1[:, :], t1[:, :], h1w[:, :])
            nc.vector.tensor_add(t1[:, :], t1[:, :], b2_sb[:, :])
            nc.sync.dma_start(out=out_d[w, :, :], in_=t1[:, :])

        warmup()
        g1_phase()
        with (
            tc.tile_pool(name="e1", bufs=6) as ep1,
            tc.tile_pool(name="e1p", bufs=2, space="PSUM") as p21,
        ):
            agg_phase(1, epi_cb=lambda w: emit_epi1(w, ep1, p21))
        with tc.tile_pool(name="e2", bufs=6) as ep2:
            agg_phase(2, epi_cb=lambda w: emit_epi2(w, ep2))
        ctx.close()

    nc.compile()
    return nc


# --------------------------------------------------------------------------
# Entry point
# --------------------------------------------------------------------------
def kernel(x, edge_index, W1, b1, W2, b2):
    x = np.asarray(x)
    edge_index = np.asarray(edge_index)
    N = x.shape[0]
    p, in_maps = _host_prepare(
        np.asarray(x, dtype=np.float32),
        edge_index,
        np.asarray(W1, dtype=np.float32),
        np.asarray(b1, dtype=np.float32),
        np.asarray(W2, dtype=np.float32),
        np.asarray(b2, dtype=np.float32),
    )
    nc = _build_program(p)

    if os.environ.get("GCN_SIM"):
        from concourse import bass_interp

        sim = bass_interp.MultiCoreSim(nc, N_CORES)
        for c in range(N_CORES):
            for k, v in in_maps[c].items():
                sim.cores[c].tensor(k)[:] = v
        sim.simulate(check_with_hw=False)
        outs = [sim.cores[c].mem_tensor("out") for c in range(N_CORES)]
    else:
        from concourse.bass_utils import run_bass_kernel_spmd

        res = run_bass_kernel_spmd(
            nc,
            in_maps,
            list(range(N_CORES)),
            trace=bool(os.environ.get("GCN_TRACE")),
        )
        kernel.last_result = res
        outs = [res.results[c]["out"] for c in range(N_CORES)]

    full = np.concatenate(
        [
            np.asarray(o, dtype=np.float32).reshape(p.SH, D)[
                : min(p.shard, N - c * p.shard)
            ]
            for c, o in enumerate(outs)
        ],
        axis=0,
    )
    return full.astype(np.float32)
